# revision 4
# baseline (speedup 1.0000x reference)
"""FFTNet Trainium2 Bass kernel.

Sharding: 8 cores = batch(4) x time-halves(2). Each core computes 8192 output
timesteps of one batch element from a 10240-wide window of the left-padded
input (8192 + 2047 halo + 1). No inter-core communication.

Per-core: activations live in one in-place SBUF buffer (2 x [128, 10240] f32,
channel groups on partitions, time on free dim). Each of the 11 FFTNet layers
is z = Wl@h[:, j] + Wr@h[:, j+shift] (+cond at layer 0), a = relu(z + b1),
h' = relu(Wp@a + b2), with h'[j] written right-aligned at buffer column
j + shift. Processing chunks right-to-left with right-aligned writes makes all
cross-chunk in-place hazards vanish (writes land exactly on the current
chunk's right-operand read range, strictly right of every later access).

Matmuls run as float32r (FP22, 1 cycle/row at N>=256) with K=128 tiles
accumulating in PSUM; bias+relu fused via ScalarE activation (stage 1) and
VectorE tensor_scalar add+max (stage 2).
"""

import sys

if "/opt/trn_rl_repo" not in sys.path:
    sys.path.insert(0, "/opt/trn_rl_repo")

import numpy as np

B, C, T, CIN, LAYERS = 4, 256, 16384, 80, 11
RF = 2 ** LAYERS  # 2048
SIL = 127
NCORES = 8
TSPLIT = 2
CHUNK_T = T // TSPLIT          # 8192 outputs per core
LIN = CHUNK_T + RF             # 10240 input window per core
NT = 512                       # time-tile (one PSUM bank of fp32)

SHIFTS = [RF >> (i + 1) for i in range(LAYERS)]  # 1024 .. 1
OFF = [0]
for _s in SHIFTS:
    OFF.append(OFF[-1] + _s)   # OFF[11] == 2047

_CACHED_NC = None


def build_program(reps: int = 1, use_loop: bool = False):
    """Build the per-core Bacc program. reps>1 repeats the whole computation
    (for timing); use_loop wraps it in a hardware For_i loop."""
    import concourse.bass as bass  # noqa: F401
    import concourse.tile as tile
    from concourse import bacc, mybir

    f32 = mybir.dt.float32
    f32r = mybir.dt.float32r
    AF = mybir.ActivationFunctionType
    ALU = mybir.AluOpType

    nc = bacc.Bacc("TRN2", target_bir_lowering=False, debug=False)
    hin = nc.dram_tensor("hin", [C, LIN], f32r, kind="ExternalInput").ap()
    cin = nc.dram_tensor("cin", [CIN, LIN], f32r, kind="ExternalInput").ap()
    wl = nc.dram_tensor("wl", [LAYERS, C, C], f32r, kind="ExternalInput").ap()
    wr = nc.dram_tensor("wr", [LAYERS, C, C], f32r, kind="ExternalInput").ap()
    wp = nc.dram_tensor("wp", [LAYERS, C, C], f32r, kind="ExternalInput").ap()
    wc = nc.dram_tensor("wc", [2, CIN, C], f32r, kind="ExternalInput").ap()
    b1 = nc.dram_tensor("b1", [LAYERS, 128, 2], f32, kind="ExternalInput").ap()
    b2 = nc.dram_tensor("b2", [LAYERS, 128, 2], f32, kind="ExternalInput").ap()
    wlast = nc.dram_tensor("wlast", [C, C], f32r, kind="ExternalInput").ap()
    blast = nc.dram_tensor("blast", [128, 2], f32, kind="ExternalInput").ap()
    out = nc.dram_tensor("out", [C, CHUNK_T], f32, kind="ExternalOutput").ap()

    with tile.TileContext(nc) as tc:
        from contextlib import ExitStack

        with ExitStack() as ctx:
            big = ctx.enter_context(tc.tile_pool(name="big", bufs=1))
            wpool = ctx.enter_context(tc.tile_pool(name="wts", bufs=2))
            apool = ctx.enter_context(tc.tile_pool(name="act", bufs=3))
            opool = ctx.enter_context(tc.tile_pool(name="out", bufs=3))
            pspool = ctx.enter_context(
                tc.tile_pool(name="ps", bufs=2, space="PSUM")
            )

            def body(_iv=None):
                h = [big.tile([128, LIN], f32r, tag=f"h{g}", name=f"h{g}") for g in range(2)]
                csb = big.tile([CIN, LIN], f32r, tag="c", name="csb")
                # Load inputs right-to-left so layer 0 (which also runs
                # right-to-left) can start before the whole window lands.
                NL = 4
                wld = LIN // NL
                for li in reversed(range(NL)):
                    sl = slice(li * wld, (li + 1) * wld)
                    for g in range(2):
                        nc.sync.dma_start(
                            out=h[g][:, sl], in_=hin[g * 128:(g + 1) * 128, sl]
                        )
                    nc.sync.dma_start(out=csb[:, sl], in_=cin[:, sl])

                # conditioning weights: [:, 0:256]=Wcl^T, [:, 256:512]=Wcr^T
                wcs = wpool.tile([CIN, 2 * C], f32r, tag="wc", name="wcs")
                nc.sync.dma_start(out=wcs[:, 0:C], in_=wc[0])
                nc.sync.dma_start(out=wcs[:, C:2 * C], in_=wc[1])

                for i in range(LAYERS):
                    wt = {}
                    for nm, src in (("l", wl), ("r", wr), ("p", wp)):
                        for kt in range(2):
                            t = wpool.tile([128, C], f32r, tag=f"w{nm}{kt}", name=f"w{nm}{kt}")
                            nc.sync.dma_start(
                                out=t[:, :],
                                in_=src[i, kt * 128:(kt + 1) * 128, :],
                            )
                            wt[nm, kt] = t
                    b1t = wpool.tile([128, 2], f32, tag="b1", name="b1t")
                    nc.sync.dma_start(out=b1t[:, :], in_=b1[i])
                    b2t = wpool.tile([128, 2], f32, tag="b2", name="b2t")
                    nc.sync.dma_start(out=b2t[:, :], in_=b2[i])

                    sh = SHIFTS[i]
                    lout = LIN - OFF[i + 1]
                    nchunks = (lout + NT - 1) // NT
                    for ci in range(nchunks):
                        end = lout - ci * NT
                        start = max(0, end - NT)
                        w = end - start
                        if w % 2:
                            # fp32r matmul needs an even free-dim count; widen
                            # one column left (stale-but-finite input, and the
                            # extra output column is never read downstream).
                            start -= 1
                            w += 1
                            assert OFF[i] + start >= 0
                        rd1 = OFF[i] + start
                        rd2 = rd1 + sh  # == write position
                        zt = [
                            pspool.tile([128, NT], f32, tag=f"z{mt}", name=f"z{mt}")
                            for mt in range(2)
                        ]
                        for mt in range(2):
                            msl = slice(mt * 128, (mt + 1) * 128)
                            seq = [
                                (wt["l", 0][:, msl], h[0][:, rd1:rd1 + w]),
                                (wt["l", 1][:, msl], h[1][:, rd1:rd1 + w]),
                                (wt["r", 0][:, msl], h[0][:, rd2:rd2 + w]),
                                (wt["r", 1][:, msl], h[1][:, rd2:rd2 + w]),
                            ]
                            if i == 0:
                                seq += [
                                    (wcs[:, msl], csb[:, rd1:rd1 + w]),
                                    (
                                        wcs[:, C + mt * 128:C + mt * 128 + 128],
                                        csb[:, rd2:rd2 + w],
                                    ),
                                ]
                            for j, (lhsT, rhs) in enumerate(seq):
                                nc.tensor.matmul(
                                    zt[mt][:, :w],
                                    lhsT,
                                    rhs,
                                    start=(j == 0),
                                    stop=(j == len(seq) - 1),
                                )
                        at = [
                            apool.tile([128, NT], f32r, tag=f"a{mt}", name=f"a{mt}")
                            for mt in range(2)
                        ]
                        for mt in range(2):
                            nc.scalar.activation(
                                at[mt][:, :w],
                                zt[mt][:, :w],
                                AF.Relu,
                                bias=b1t[:, mt:mt + 1],
                            )
                        z2 = [
                            pspool.tile([128, NT], f32, tag=f"y{mt}", name=f"y{mt}")
                            for mt in range(2)
                        ]
                        for mt in range(2):
                            msl = slice(mt * 128, (mt + 1) * 128)
                            nc.tensor.matmul(
                                z2[mt][:, :w],
                                wt["p", 0][:, msl],
                                at[0][:, :w],
                                start=True,
                                stop=False,
                            )
                            nc.tensor.matmul(
                                z2[mt][:, :w],
                                wt["p", 1][:, msl],
                                at[1][:, :w],
                                start=False,
                                stop=True,
                            )
                        for mt in range(2):
                            nc.vector.tensor_scalar(
                                out=h[mt][:, rd2:rd2 + w],
                                in0=z2[mt][:, :w],
                                scalar1=b2t[:, mt:mt + 1],
                                scalar2=0.0,
                                op0=ALU.add,
                                op1=ALU.max,
                            )

                # final channel-wise linear (no relu)
                wlt = []
                for kt in range(2):
                    t = wpool.tile([128, C], f32r, tag=f"wlast{kt}", name=f"wlast{kt}")
                    nc.sync.dma_start(
                        out=t[:, :], in_=wlast[kt * 128:(kt + 1) * 128, :]
                    )
                    wlt.append(t)
                blt = wpool.tile([128, 2], f32, tag="blast", name="blt")
                nc.sync.dma_start(out=blt[:, :], in_=blast[:, :])
                for ci in range(CHUNK_T // NT):
                    st = ci * NT
                    rd = OFF[LAYERS] + st
                    zt = [
                        pspool.tile([128, NT], f32, tag=f"z{mt}", name=f"z{mt}")
                        for mt in range(2)
                    ]
                    for mt in range(2):
                        msl = slice(mt * 128, (mt + 1) * 128)
                        nc.tensor.matmul(
                            zt[mt][:, :],
                            wlt[0][:, msl],
                            h[0][:, rd:rd + NT],
                            start=True,
                            stop=False,
                        )
                        nc.tensor.matmul(
                            zt[mt][:, :],
                            wlt[1][:, msl],
                            h[1][:, rd:rd + NT],
                            start=False,
                            stop=True,
                        )
                    ot = [
                        opool.tile([128, NT], f32, tag=f"o{mt}", name=f"o{mt}")
                        for mt in range(2)
                    ]
                    # split the two bias-adds across ScalarE and VectorE
                    nc.scalar.activation(
                        ot[0][:, :], zt[0][:, :], AF.Identity,
                        bias=blt[:, 0:1],
                    )
                    nc.vector.tensor_scalar(
                        out=ot[1][:, :], in0=zt[1][:, :],
                        scalar1=blt[:, 1:2], scalar2=None,
                        op0=ALU.add,
                    )
                    for mt in range(2):
                        nc.sync.dma_start(
                            out=out[mt * 128:(mt + 1) * 128, st:st + NT],
                            in_=ot[mt][:, :],
                        )

            if use_loop and reps > 1:
                with tc.For_i(0, reps, 1):
                    body()
            else:
                for _ in range(reps):
                    body()

    nc.compile()
    return nc


def prepare_in_maps(x, c, Wl, bl, Wr, br, Wp, bp, Wcl, bcl, Wcr, bcr,
                    Wlast, blast):
    f = lambda a: np.ascontiguousarray(np.asarray(a, dtype=np.float32))
    x, c = f(x), f(c)
    Wl, bl, Wr, br, Wp, bp = f(Wl), f(bl), f(Wr), f(br), f(Wp), f(bp)
    Wcl, bcl, Wcr, bcr = f(Wcl), f(bcl), f(Wcr), f(bcr)
    Wlast, blast = f(Wlast), f(blast)

    hpad = np.zeros((B, C, T + RF), np.float32)
    hpad[:, SIL, :RF] = 1.0
    hpad[:, :, RF:] = x
    cpad = np.zeros((B, CIN, T + RF), np.float32)
    cpad[:, :, RF:] = c

    wlT = np.ascontiguousarray(Wl.transpose(0, 2, 1))   # [i, cin, cout]
    wrT = np.ascontiguousarray(Wr.transpose(0, 2, 1))
    wpT = np.ascontiguousarray(Wp.transpose(0, 2, 1))
    wcT = np.ascontiguousarray(np.stack([Wcl.T, Wcr.T]))  # (2, 80, 256)
    b1v = bl + br
    b1v[0] += bcl + bcr
    b1d = np.ascontiguousarray(
        b1v.reshape(LAYERS, 2, 128).transpose(0, 2, 1))   # (11, 128, 2)
    b2d = np.ascontiguousarray(bp.reshape(LAYERS, 2, 128).transpose(0, 2, 1))
    wlastT = np.ascontiguousarray(Wlast.T)
    blastD = np.ascontiguousarray(blast.reshape(2, 128).T)  # (128, 2)

    in_maps = []
    for k in range(NCORES):
        bi, s = divmod(k, TSPLIT)
        t0 = s * CHUNK_T
        in_maps.append(dict(
            hin=np.ascontiguousarray(hpad[bi, :, t0:t0 + LIN]),
            cin=np.ascontiguousarray(cpad[bi, :, t0:t0 + LIN]),
            wl=wlT, wr=wrT, wp=wpT, wc=wcT, b1=b1d, b2=b2d,
            wlast=wlastT, blast=blastD,
        ))
    return in_maps


def assemble_output(results):
    out = np.empty((B, C, T), np.float32)
    for k in range(NCORES):
        bi, s = divmod(k, TSPLIT)
        out[bi, :, s * CHUNK_T:(s + 1) * CHUNK_T] = results[k]["out"]
    return out


def kernel(**inputs) -> np.ndarray:
    global _CACHED_NC
    from concourse.bass_utils import run_bass_kernel_spmd

    in_maps = prepare_in_maps(**inputs)
    if _CACHED_NC is None:
        _CACHED_NC = build_program()
    res = run_bass_kernel_spmd(_CACHED_NC, in_maps, core_ids=list(range(NCORES)))
    return assemble_output(res.results)
